# revision 77
# baseline (speedup 1.0000x reference)
"""Trainium2 Bass kernel for a Lorentz RGCN message-passing layer.

Strategy (8 NeuronCores, SPMD, no collectives):
  - Nodes are range-partitioned by destination: core c owns 6272 dst nodes.
    Each core processes all edges whose dst it owns and writes a disjoint
    slice of the output.
  - Within a core, its 6272 nodes are PERMUTED into 49 windows of 128 by
    LPT (longest-processing-time) bin packing on in-degree, so every
    window holds <= 17*128 = 2176 edges (vs 18 tiles for the naive
    contiguous split).  One dma_gather of 2176 pair-indices per window.
  - Since NUM_BASES == D (SI=SO=1) the relation transform is elementwise:
    msg = h_tangent[src] * weight[etype] + rel_emb[etype].  The
    weight/rel_emb rows are expanded per edge-slot ON THE HOST into a
    [128, NTILES, 256] f16 table streamed with plain contiguous DMA
    (no per-edge gather descriptors for the tables).
  - h_tangent lives in DRAM as a f16 table; rows are fetched in PAIRS
    (pair index < 25088 fits the gather's int16 index ucode) and the
    right half is selected by parity via copy_predicated.
  - Per-edge exp0/to_lorentz scalars are batched per GROUP of 7 windows
    ([128, 119] ops) to amortize per-instruction overhead.
  - Segment sums: TensorEngine one-hot matmuls (f16) into a PSUM
    [128 nodes x 130] accumulator per window; per-edge centroid weight
    is folded into the one-hot.
  - Self-loop matmuls use a DMA-transposed f16 copy of the core's own
    h_tangent block; per-node epilogue is batched per group.
"""

import sys

sys.path.insert(0, "/opt/trn_rl_repo")

import numpy as np
import ml_dtypes

import concourse.bass as bass
import concourse.bacc as bacc
import concourse.mybir as mybir
from concourse.tile import TileContext

# ---------------------------------------------------------------- constants
NCORES = 8
N = 50000
E = 800000
D = 128
R = 230
C = 0.01
SC = 0.1  # sqrt(C)
EPS = 1e-7

NPC = 6272                 # nodes per core = 49 windows * 128
NW = 49                    # windows per core
TPW = 18                   # tiles per window: 9 even-parity + 9 odd-parity
HPW = TPW // 2             # 9 tiles per parity half
EPP = HPW * 128            # 1152 edge slots per parity half
EPW = TPW * 128            # 2304 edge slots per window
NTILES = NW * TPW          # 882
ESLOT = NTILES * 128       # 112896 edge slots per core
NROT = NCORES * NPC        # 50176 rows in the (rolled, padded) h table
GRP = 7                    # max windows per group (chain batching)
GROUPS = [7, 7, 7, 7, 7, 7, 5, 2]   # last groups smaller -> short tail
NGRP = len(GROUPS)
IDXW = EPW // 16           # 144 index columns per window (72 even + 72 odd)
IDXC = NW * IDXW           # 6664

f32 = mybir.dt.float32
f16 = mybir.dt.float16
f8 = mybir.dt.float8e4
i16 = mybir.dt.int16
i32 = mybir.dt.int32
i8 = mybir.dt.int8
OP = mybir.AluOpType
AF = mybir.ActivationFunctionType

SUP = 14                   # rows-per-partition per phase-A supertile
NSUP = NROT // (SUP * 128)  # 28


# ------------------------------------------------------------ device program
_PROGRAM = None


def _build_program():
    nc = bacc.Bacc("TRN2", target_bir_lowering=False, debug=False)

    h_roll = nc.declare_dram_parameter("h_roll", [NROT, D], f16, isOutput=False)
    h_perm = nc.declare_dram_parameter("h_perm", [NPC, D], f16, isOutput=False)
    wr_e = nc.declare_dram_parameter("wr_e", [128, NTILES, 2 * D], f16, isOutput=False)
    lw_d = nc.declare_dram_parameter("lw", [D, D], f16, isOutput=False)
    ev_d = nc.declare_dram_parameter("ev", [D, D], f16, isOutput=False)
    norm_d = nc.declare_dram_parameter("norm_c", [NPC, 1], f32, isOutput=False)
    deg_d = nc.declare_dram_parameter("deg_c", [NPC, 1], f32, isOutput=False)
    idxh_d = nc.declare_dram_parameter("idx_h", [128, IDXC], i16, isOutput=False)
    cnt_d = nc.declare_dram_parameter("cnt", [1, 2 * NW], i32, isOutput=False)
    oneh_d = nc.declare_dram_parameter("oneh", [128, NTILES, 128], f8, isOutput=False)
    out_d = nc.declare_dram_parameter("out", [NPC, D], f32, isOutput=True)
    htab = nc.dram_tensor("htab", [NROT, D], f16)
    htp = nc.dram_tensor("htp", [NPC, D], f16)

    with TileContext(nc) as tc:
        with (
            tc.tile_pool(name="persist", bufs=1) as pp,
            tc.tile_pool(name="consts", bufs=1) as cp,
        ):
            hT = pp.tile([128, NPC], f16)          # h_tangent^T of own nodes
            cnt_sb = pp.tile([1, 2 * NW], i32)
            hb_bufs = [
                pp.tile([128, TPW, D], f16, tag=f"hbp{k}", name=f"hbp{k}")
                for k in range(6)
            ]
            norm_sb = pp.tile([128, NW], f32)
            deg_sb = pp.tile([128, NW], f32)

            LW = cp.tile([128, D], f16)
            EV = cp.tile([128, D], f16)

            nc.sync.dma_start(
                out=norm_sb[:], in_=norm_d[:].rearrange("(w p) o -> p (w o)", p=128)
            )
            nc.sync.dma_start(
                out=deg_sb[:], in_=deg_d[:].rearrange("(w p) o -> p (w o)", p=128)
            )
            nc.sync.dma_start(out=LW[:], in_=lw_d[:])
            nc.sync.dma_start(out=EV[:], in_=ev_d[:])
            nc.sync.dma_start(out=cnt_sb[:], in_=cnt_d[:])
            for k in range(6):
                nc.gpsimd.memset(hb_bufs[k][:], 0.0)

            # ---------------- phase A: h_tangent table (log0 of h_roll) ----
            # (p t) layout: supertile s covers rows [s*1792, (s+1)*1792),
            # partition p holds rows s*1792 + p*14 .. +13 -> 3.5KB DMA descs.
            with tc.tile_pool(name="phA", bufs=3) as pa:
                # (src_dram, dst_dram, row0, nrows_per_partition)
                a_jobs = [(h_roll, htab, s * SUP * 128, SUP) for s in range(NSUP)]
                a_jobs += [(h_perm, htp, s * SUP * 128, SUP) for s in range(3)]
                a_jobs += [(h_perm, htp, 3 * SUP * 128, 7)]
                for src_t, dst_t, r0, sup in a_jobs:
                    xin = pa.tile([128, SUP, D], f16, tag="xin", name="xin")[
                        :, 0:sup, :
                    ]
                    nc.sync.dma_start(
                        out=xin,
                        in_=src_t[r0 : r0 + sup * 128, :].rearrange(
                            "(p t) d -> p t d", t=sup
                        ),
                    )
                    # artanh(sc*n)/(sc*n) = 1 + v/3 + v^2/5 + O(v^3), v = C*n2.
                    # |h| <= ~0.6 so v <= 0.004: quadratic is exact to 1e-8.
                    sqv = pa.tile([128, SUP, D], f16, tag="sqv", name="sqv")[
                        :, 0:sup, :
                    ]
                    nc.scalar.activation(sqv, xin, AF.Square, scale=SC)
                    v = pa.tile([128, SUP], f16, tag="v", name="v")[:, 0:sup]
                    with nc.allow_low_precision("f16 n2 accum, rel ~1e-3"):
                        nc.vector.reduce_sum(
                            out=v, in_=sqv, axis=mybir.AxisListType.X
                        )
                    tq = pa.tile([128, SUP], f32, tag="tq", name="tq")[:, 0:sup]
                    nc.vector.tensor_scalar(
                        out=tq, in0=v, scalar1=0.2, scalar2=1.0 / 3.0,
                        op0=OP.mult, op1=OP.add,
                    )
                    vt = pa.tile([128, SUP], f32, tag="vt", name="vt")[:, 0:sup]
                    nc.vector.tensor_tensor(
                        out=vt, in0=v, in1=tq, op=OP.mult
                    )
                    scl = pa.tile([128, SUP], f32, tag="scl", name="scl")[:, 0:sup]
                    nc.vector.tensor_scalar(
                        out=scl, in0=vt, scalar1=1.0, scalar2=None, op0=OP.add
                    )
                    hts = pa.tile([128, SUP, D], f16, tag="hts", name="hts")[
                        :, 0:sup, :
                    ]
                    scl_bc = bass.AP(
                        scl.tensor, scl.offset, [scl.ap[0], scl.ap[1], [0, D]]
                    )
                    nc.vector.tensor_tensor(
                        out=hts, in0=xin, in1=scl_bc, op=OP.mult
                    )
                    nc.sync.dma_start(
                        out=dst_t[r0 : r0 + sup * 128, :].rearrange(
                            "(p t) d -> p t d", t=sup
                        ),
                        in_=hts,
                    )

            tc.strict_bb_all_engine_barrier()
            # transposed copy of own nodes' h_tangent for self-loop matmuls
            nc.sync.dma_start_transpose(hT[:], htp[:])

            # ---------------- phase B/C/D: edges, segments, epilogue -------
            htab_pairs = htab[:].rearrange("(a b) d -> a (b d)", b=2)
            nreg = nc.gpsimd.to_reg(EPP)
            with (
                tc.tile_pool(name="pid", bufs=4) as pid,
                tc.tile_pool(name="pwr", bufs=2) as pwr,
                tc.tile_pool(name="poh", bufs=4) as poh,
                tc.tile_pool(name="scr", bufs=2) as scr,
                tc.tile_pool(name="pg", bufs=2) as pg,
                tc.tile_pool(name="prh", bufs=2) as prh,
                tc.tile_pool(name="pc", bufs=2) as pc,
                tc.tile_pool(name="psum", bufs=2, space="PSUM") as psp,
            ):
                def TS(dst, src, s1, s2=None, o0=OP.mult, o1=None):
                    if o1 is None:
                        nc.vector.tensor_scalar(
                            out=dst, in0=src, scalar1=s1, scalar2=None, op0=o0
                        )
                    else:
                        nc.vector.tensor_scalar(
                            out=dst, in0=src, scalar1=s1, scalar2=s2,
                            op0=o0, op1=o1,
                        )

                def TT(dst, a, b, op):
                    nc.vector.tensor_tensor(out=dst, in0=a, in1=b, op=op)

                def gather_stage(g, j, st):
                    w = st["w0"] + j
                    idx_t = pid.tile([128, IDXW], i16, tag="idx")
                    nc.sync.dma_start(
                        out=idx_t[:], in_=idxh_d[:, IDXW * w : IDXW * (w + 1)]
                    )
                    hb = hb_bufs[w % 6]
                    nc.gpsimd.dma_gather(
                        out_ap=hb[:, 0:HPW, :], in_ap=htab_pairs[:, 0:128],
                        idxs_ap=idx_t[:, 0 : IDXW // 2],
                        num_idxs=EPP, num_idxs_reg=nreg, elem_size=D,
                        elem_step=2 * D, single_packet=False,
                    )
                    nc.gpsimd.dma_gather(
                        out_ap=hb[:, HPW:TPW, :], in_ap=htab_pairs[:, 128:256],
                        idxs_ap=idx_t[:, IDXW // 2 : IDXW],
                        num_idxs=EPP, num_idxs_reg=nreg, elem_size=D,
                        elem_step=2 * D, single_packet=False,
                    )
                    wrb = pwr.tile([128, TPW, 2 * D], f16, tag="wrb")
                    nc.sync.dma_start(
                        out=wrb[:], in_=wr_e[:, TPW * w : TPW * (w + 1), :]
                    )
                    rhs_w = prh.tile([128, TPW, 130], f16, tag=f"rhs{j}")
                    st["rhs"].append(rhs_w)
                    msg = rhs_w[:, :, 0:128]
                    nc.vector.tensor_tensor(
                        out=msg, in0=hb[:], in1=wrb[:, :, 0:128], op=OP.mult
                    )
                    nc.vector.tensor_tensor(
                        out=msg, in0=msg, in1=wrb[:, :, 128:256], op=OP.add
                    )
                    # u = C*n2 (C folded into Square's scale); u <= ~0.06
                    sqv = scr.tile([128, TPW, D], f16, tag="sqv")
                    nc.scalar.activation(sqv[:], msg, AF.Square, scale=SC)
                    with nc.allow_low_precision("f16 n2 accum, rel ~1e-3"):
                        nc.vector.reduce_sum(
                            out=st["ug"][:, TPW * j : TPW * (j + 1)], in_=sqv[:],
                            axis=mybir.AxisListType.X,
                        )

                def chain_stage(g, st):
                    gs = st["gs"]
                    ncol = gs * TPW
                    # exp0/to_lorentz per-edge scalars as polynomials in u:
                    #   P = tanh(s)/s = 1 - u/3 + 2u^2/15 - 17u^3/315
                    #   dn = 1 - u*P^2;  sxi = 2P/dn;  dx = 20*u*P^2/dn
                    # (s = sqrt(C)*|msg|, u = s^2; cubic exact to ~3e-7)
                    def PCT(tag):
                        return pc.tile(
                            [128, GRP * TPW], f32, tag=tag, name=tag
                        )[:, 0:ncol]

                    ug = st["ug"][:, 0:ncol]
                    ta = PCT("ta")
                    TS(ta, ug, -1.0 / 3.0, 1.0, OP.mult, OP.add)
                    tb = PCT("tb")
                    TS(tb, ug, -17.0 / 315.0, 2.0 / 15.0, OP.mult, OP.add)
                    u2 = PCT("u2")
                    TT(u2, ug, ug, OP.mult)
                    u2tb = PCT("u2tb")
                    TT(u2tb, u2, tb, OP.mult)
                    P = PCT("P")
                    TT(P, ta, u2tb, OP.add)
                    P2 = PCT("P2")
                    TT(P2, P, P, OP.mult)
                    q = PCT("q")
                    TT(q, ug, P2, OP.mult)
                    dn = PCT("dn")
                    TS(dn, q, -1.0, 1.0, OP.mult, OP.add)
                    rd = PCT("rd")
                    nc.vector.reciprocal(rd, dn)
                    sxi = PCT("sxi")
                    nc.vector.scalar_tensor_tensor(
                        out=sxi, in0=P, scalar=2.0, in1=rd,
                        op0=OP.mult, op1=OP.mult,
                    )
                    dx = PCT("dx")
                    nc.vector.scalar_tensor_tensor(
                        out=dx, in0=q, scalar=2.0 / SC, in1=rd,
                        op0=OP.mult, op1=OP.mult,
                    )
                    st["sxi"], st["dx"] = sxi, dx

                def post_scale(g, j, st):
                    rhs_w = st["rhs"][j]
                    msg = rhs_w[:, :, 0:128]
                    nc.vector.tensor_tensor(
                        out=msg, in0=msg,
                        in1=st["sxi"][:, TPW * j : TPW * (j + 1)].to_broadcast(
                            [128, TPW, 128]
                        ),
                        op=OP.mult,
                    )
                    nc.scalar.copy(
                        rhs_w[:, :, 128], st["dx"][:, TPW * j : TPW * (j + 1)]
                    )

                def post_mm(g, j, st):
                    w = st["w0"] + j
                    rhs_w = st["rhs"][j]
                    oh_w = poh.tile([128, TPW, 128], f8, tag="oh")
                    nc.sync.dma_start(
                        out=oh_w[:], in_=oneh_d[:, TPW * w : TPW * (w + 1), :]
                    )
                    ps = psp.tile([128, 129], f32, tag="ps")
                    for t in range(TPW):
                        nc.tensor.matmul(
                            ps[:], oh_w[:, t, :], rhs_w[:, t, 0:129],
                            start=(t == 0), stop=(t == TPW - 1),
                        )
                    # phase C
                    nc.scalar.copy(st["Sg"][:, j, :], ps[:])
                    sq2 = scr.tile([128, 128], f16, tag="sq2")
                    nc.scalar.activation(
                        sq2[:], st["Sg"][:, j, 0:128], AF.Square,
                        accum_out=st["s2r"][:, j : j + 1],
                    )
                    lp = psp.tile([128, 128], f32, tag="lp")
                    nc.tensor.matmul(
                        lp[:], hT[:, 128 * w : 128 * (w + 1)], LW[:],
                        start=True, stop=True,
                    )
                    ep = psp.tile([128, 128], f32, tag="ep")
                    nc.tensor.matmul(
                        ep[:], hT[:, 128 * w : 128 * (w + 1)], EV[:],
                        start=True, stop=True,
                    )
                    mk = scr.tile([128, 1], i8, tag="mk")
                    nc.vector.tensor_scalar(
                        out=mk[:], in0=deg_sb[:, w : w + 1], scalar1=0.0,
                        scalar2=None, op0=OP.is_gt,
                    )
                    nc.scalar.copy(st["hng"][:, j, :], ep[:])
                    nc.vector.copy_predicated(
                        out=st["hng"][:, j, :], mask=mk[:].to_broadcast([128, 128]),
                        data=lp[:],
                    )

                def d_stage(g, st):
                    gs, w0 = st["gs"], st["w0"]
                    Sg = st["Sg"][:, 0:gs, :]
                    hng = st["hng"][:, 0:gs, :]
                    s2r = st["s2r"][:, 0:gs]

                    def B(tag):
                        return pc.tile([128, GRP], f32, tag=tag, name=tag)[:, 0:gs]

                    nrm = norm_sb[:, w0 : w0 + gs]
                    deg = deg_sb[:, w0 : w0 + gs]
                    Sdx = Sg[:, :, 128]
                    q = B("Dq")
                    TT(q, nrm, deg, OP.mult)
                    qq = B("Dqq")
                    TS(qq, q, 1e-6, o0=OP.add)
                    rq = B("Drq")
                    nc.vector.reciprocal(rq, qq)
                    fac = B("Dfac")
                    TT(fac, nrm, rq, OP.mult)
                    S0 = B("DS0")
                    nc.vector.scalar_tensor_tensor(
                        out=S0, in0=deg, scalar=1.0 / SC, in1=Sdx,
                        op0=OP.mult, op1=OP.add,
                    )
                    mu0 = B("Dmu0")
                    TT(mu0, S0, fac, OP.mult)
                    f2 = B("Df2")
                    TT(f2, fac, fac, OP.mult)
                    s0sq = B("Ds0sq")
                    TT(s0sq, S0, S0, OP.mult)
                    s2a = B("Ds2a")
                    TT(s2a, s2r, s0sq, OP.add)
                    s2 = B("Ds2")
                    TT(s2, s2a, f2, OP.mult)
                    m0s = B("Dm0s")
                    TT(m0s, mu0, mu0, OP.mult)
                    mink = B("Dmink")
                    nc.vector.scalar_tensor_tensor(
                        out=mink, in0=m0s, scalar=-2.0, in1=s2,
                        op0=OP.mult, op1=OP.add,
                    )
                    ab = B("Dab")
                    nc.scalar.activation(ab, mink, AF.Abs)
                    am = B("Dam")
                    TS(am, ab, EPS, o0=OP.max)
                    sqm = B("Dsqm")
                    nc.scalar.activation(sqm, am, AF.Sqrt)
                    rr = B("Drr")
                    nc.vector.reciprocal(rr, sqm)
                    c0 = B("Dc0")
                    nc.vector.scalar_tensor_tensor(
                        out=c0, in0=mu0, scalar=1.0 / SC, in1=rr,
                        op0=OP.mult, op1=OP.mult,
                    )
                    pd = B("Dpd")
                    TS(pd, c0, SC, 1.0, OP.mult, OP.add)
                    pdc = B("Dpdc")
                    TS(pdc, pd, EPS, o0=OP.max)
                    rpd = B("Drpd")
                    nc.vector.reciprocal(rpd, pdc)
                    s_y = B("Dsy")
                    nc.vector.scalar_tensor_tensor(
                        out=s_y, in0=rr, scalar=1.0 / SC, in1=rpd,
                        op0=OP.mult, op1=OP.mult,
                    )
                    sp2 = B("Dsp2")
                    TT(sp2, s2, m0s, OP.subtract)
                    y2 = B("Dy2")
                    TT(y2, s_y, s_y, OP.mult)
                    ny2 = B("Dny2")
                    TT(ny2, y2, sp2, OP.mult)
                    nyr = B("Dnyr")
                    nc.scalar.activation(nyr, ny2, AF.Sqrt)
                    ny = B("Dny")
                    TS(ny, nyr, EPS, o0=OP.max)
                    v = B("Dv")
                    TS(v, ny, SC, 1.0 - EPS, OP.mult, OP.min)
                    la = B("Dla")
                    nc.scalar.activation(la, v, AF.Ln, bias=1.0, scale=1.0)
                    lb = B("Dlb")
                    nc.scalar.activation(lb, v, AF.Ln, bias=1.0, scale=-1.0)
                    df = B("Ddf")
                    TT(df, la, lb, OP.subtract)
                    rny = B("Drny")
                    nc.vector.reciprocal(rny, ny)
                    t1 = B("Dt1")
                    nc.vector.scalar_tensor_tensor(
                        out=t1, in0=df, scalar=0.5 / SC, in1=rny,
                        op0=OP.mult, op1=OP.mult,
                    )
                    k1 = B("Dk1")
                    TT(k1, t1, s_y, OP.mult)
                    hfac = B("Dhfac")
                    TT(hfac, k1, fac, OP.mult)

                    # big [128, gs, 128] ops
                    tmp = scr.tile([128, GRP, D], f32, tag="Dtmp", name="Dtmp")[:, 0:gs, :]
                    nc.vector.tensor_tensor(
                        out=tmp, in0=Sg[:, :, 0:128],
                        in1=hfac.to_broadcast([128, gs, 128]), op=OP.mult
                    )
                    nc.vector.tensor_scalar(
                        out=tmp, in0=tmp, scalar1=10.0, scalar2=-10.0,
                        op0=OP.min, op1=OP.max,
                    )
                    nc.vector.tensor_tensor(
                        out=hng, in0=tmp, in1=hng, op=OP.add
                    )
                    nc.vector.tensor_scalar(
                        out=hng, in0=hng, scalar1=10.0, scalar2=-10.0,
                        op0=OP.min, op1=OP.max,
                    )
                    sqd = scr.tile([128, GRP, D], f16, tag="Dsqd", name="Dsqd")[:, 0:gs, :]
                    nc.scalar.activation(sqd, hng, AF.Square)
                    ne2 = pc.tile([128, GRP], f16, tag="Dne2", name="Dne2")[:, 0:gs]
                    with nc.allow_low_precision("f16 ne2 accum, rel ~1e-3"):
                        nc.vector.reduce_sum(
                            out=ne2, in_=sqd, axis=mybir.AxisListType.X
                        )
                    nnf = B("Dnnf")
                    nc.scalar.activation(nnf, ne2, AF.Sqrt)
                    nnc = B("Dnnc")
                    TS(nnc, nnf, EPS, o0=OP.max)
                    thf = B("Dthf")
                    nc.scalar.activation(thf, nnc, AF.Tanh, scale=SC)
                    rnf = B("Drnf")
                    nc.vector.reciprocal(rnf, nnc)
                    sf = B("Dsf")
                    nc.vector.scalar_tensor_tensor(
                        out=sf, in0=thf, scalar=1.0 / SC, in1=rnf,
                        op0=OP.mult, op1=OP.mult,
                    )
                    nc.vector.tensor_tensor(
                        out=hng, in0=hng,
                        in1=sf.to_broadcast([128, gs, 128]), op=OP.mult
                    )
                    r0 = w0 * 128
                    nc.sync.dma_start(
                        out=out_d[r0 : r0 + gs * 128, :].rearrange(
                            "(w p) d -> p w d", p=128
                        ),
                        in_=hng,
                    )

                # software pipeline: group g's message-building interleaves
                # with group g-1's scale/matmul/epilogue at window granularity
                # so the vector engine never runs a long burst that stalls
                # the gather chain.
                W0 = [sum(GROUPS[:k]) for k in range(NGRP)]
                prev = None
                for g in range(NGRP + 1):
                    st = None
                    if g < NGRP:
                        st = {
                            "gs": GROUPS[g],
                            "w0": W0[g],
                            "ug": pg.tile([128, GRP * TPW], f16, tag="ug", name="ug"),
                            "Sg": pg.tile([128, GRP, 129], f32, tag="Sg", name="Sg"),
                            "hng": pg.tile([128, GRP, D], f32, tag="hng", name="hng"),
                            "s2r": pg.tile([128, GRP], f32, tag="s2r", name="s2r"),
                            "rhs": [],
                        }
                    npost = prev["gs"] if prev is not None else 0
                    ngath = st["gs"] if st is not None else 0
                    for j in range(max(npost, ngath)):
                        if j < ngath:
                            gather_stage(g, j, st)
                        if j < npost:
                            post_scale(g - 1, j, prev)
                            post_mm(g - 1, j, prev)
                    if st is not None:
                        chain_stage(g, st)
                    if prev is not None:
                        d_stage(g - 1, prev)
                    prev = st
    return nc


def get_program():
    global _PROGRAM
    if _PROGRAM is None:
        _PROGRAM = _build_program()
        _PROGRAM.compile()
    return _PROGRAM


# ------------------------------------------------------------ host wrapper
def _lpt_permute(deg_e, deg_o):
    """Assign NPC nodes to NW capacity-128 windows, balancing the even- and
    odd-parity degree sums jointly (budget: EPP edges per parity per window).
    Returns p2n: position -> original local node."""
    import heapq

    deg = deg_e + deg_o
    order = np.argsort(-deg, kind="stable")
    heap = [(0, w) for w in range(NW)]
    heapq.heapify(heap)
    members = [[] for _ in range(NW)]
    le = [0] * NW
    lo = [0] * NW
    for n in order:
        tmp = []
        while True:
            key, w = heapq.heappop(heap)
            if len(members[w]) < 128:
                break
            tmp.append((key, w))
        for t in tmp:
            heapq.heappush(heap, t)
        members[w].append(n)
        le[w] += int(deg_e[n])
        lo[w] += int(deg_o[n])
        if len(members[w]) < 128:
            heapq.heappush(heap, (max(le[w], lo[w]), w))
    for w in range(NW):
        if le[w] > EPP or lo[w] > EPP:
            raise RuntimeError(f"window parity overflow: {le[w]}/{lo[w]} > {EPP}")
    p2n = np.concatenate([np.array(m, dtype=np.int64) for m in members])
    return p2n


def _preprocess(h_hyper, weight, loop_weight, evolve_loop_weight, rel_emb,
                norm, src, dst, etype):
    wrcat = np.concatenate(
        [weight.reshape(R, D), rel_emb.reshape(R, D)], axis=1
    ).astype(np.float32)
    h_pad = np.zeros((NROT, D), np.float32)
    h_pad[:N] = h_hyper
    src = src.astype(np.int64)
    dst = dst.astype(np.int64)
    core = dst // NPC

    in_maps = []
    perms = []
    for c in range(NCORES):
        m = core == c
        src_c, et_c = src[m], etype[m].astype(np.int64)
        d_loc = dst[m] - c * NPC
        rot = (src_c - c * NPC) % NROT
        par_c = (rot & 1).astype(np.int64)
        deg_e = np.bincount(d_loc[par_c == 0], minlength=NPC)
        deg_o = np.bincount(d_loc[par_c == 1], minlength=NPC)
        p2n = _lpt_permute(deg_e, deg_o)
        n2p = np.empty(NPC, np.int64)
        n2p[p2n] = np.arange(NPC)
        perms.append(p2n)

        pos_node = n2p[d_loc]
        win = pos_node >> 7
        lane = pos_node & 127

        # slot assignment: within window w, even-parity edges occupy slots
        # [w*EPW, w*EPW+EPP), odd-parity [w*EPW+EPP, (w+1)*EPW)
        key = win * 2 + par_c
        order = np.argsort(key, kind="stable")
        src_s, et_s, win_s, lane_s, par_s, rot_s = (
            src_c[order], et_c[order], win[order], lane[order], par_c[order],
            rot[order],
        )
        counts = np.bincount(win_s * 2 + par_s, minlength=2 * NW)
        if counts.max() > EPP:
            raise RuntimeError(
                f"half-window overflow: {counts.max()} > {EPP}"
            )
        offs = np.concatenate([[0], np.cumsum(counts)[:-1]])
        half = win_s * 2 + par_s
        slot = win_s * EPW + par_s * EPP + (np.arange(len(half)) - offs[half])

        # padding slots gather row-pair 0 (finite); wr=0 zeroes the message
        # and an all-zero one-hot row keeps them out of the segment sums
        pair = np.zeros(ESLOT, np.int64)
        pair[slot] = rot_s >> 1
        oneh = np.zeros((ESLOT, 128), ml_dtypes.float8_e4m3fn)
        oneh[slot, lane_s] = 1.0
        wr_s = np.zeros((ESLOT, 2 * D), np.float32)
        wr_s[slot] = wrcat[et_s]

        # index buffer: per window two wrapped halves of 72 columns each
        a2 = pair.reshape(NW, 2, EPP // 16, 16).transpose(0, 1, 3, 2)
        big = a2.reshape(NW * 2, 16, EPP // 16).transpose(1, 0, 2).reshape(16, IDXC)
        idx_h = np.tile(big, (8, 1)).astype(np.int16)

        h_roll = h_pad[(np.arange(NROT) + c * NPC) % NROT].astype(np.float16)
        h_perm = h_pad[c * NPC + p2n].astype(np.float16)

        deg = deg_e + deg_o
        n_real = min(NPC, N - c * NPC)
        norm_full = np.ones(NPC, np.float32)
        norm_full[:n_real] = norm[c * NPC : c * NPC + n_real, 0].astype(np.float32)
        norm_c = norm_full[p2n].reshape(NPC, 1)
        deg_c = deg.astype(np.float32)[p2n].reshape(NPC, 1)

        in_maps.append({
            "h_roll": h_roll,
            "h_perm": h_perm,
            "wr_e": wr_s.reshape(NTILES, 128, 2 * D).transpose(1, 0, 2)
                        .astype(np.float16),
            "lw": loop_weight.astype(np.float16),
            "ev": evolve_loop_weight.astype(np.float16),
            "norm_c": norm_c,
            "deg_c": deg_c,
            "idx_h": idx_h,
            "cnt": counts.astype(np.int32).reshape(1, 2 * NW),
            "oneh": oneh.reshape(NTILES, 128, 128).transpose(1, 0, 2).copy(),
        })
    return in_maps, perms


def run(inputs, trace=False, **kw):
    from concourse.bass_utils import run_bass_kernel_spmd

    nc = get_program()
    in_maps, perms = _preprocess(**inputs)
    res = run_bass_kernel_spmd(nc, in_maps, list(range(NCORES)), trace=trace, **kw)
    out = np.empty((N, D), np.float32)
    for c in range(NCORES):
        n_real = min(NPC, N - c * NPC)
        o = res.results[c]["out"]            # rows are permuted positions
        p2n = perms[c]
        keep = p2n < n_real
        out[c * NPC + p2n[keep]] = o[keep]
    return out, res


def kernel(**inputs) -> np.ndarray:
    out, _ = run(inputs)
    return out
